# revision 65
# baseline (speedup 1.0000x reference)
"""Multi-head self-attention on 8 trn2 NeuronCores.

Problem: x[2,2048,1024], 16 heads, depth 64; out = MHA(x) with QKV/O
projections (nn_MultiHeadSelfAttention_3341484556968).

Sharding: tensor-parallel over heads. Core c owns heads {2c, 2c+1} (128
features). All matmuls run in bf16 (full PE rate, fp32 psum accumulation).

Per core:
  - Q/K projections in T-layout ([feat, rows]), weights stationary, x
    streamed transposed in bf16. bk is dropped (softmax over keys is
    invariant to per-query constants); bv is folded into bo on the host
    (attention weights sum to 1), so only bq is applied on device.
  - The V projection runs in the opposite orientation (x^T row-chunk
    stationary, W_v^T moving), producing V directly in [keys, feat]
    layout with the softmax-denominator ones columns interleaved — no
    transpose anywhere on the V path.
  - Scores computed transposed ([k, q]); the two heads are row-packed on
    the PE via tile_position (K=64 each). exp on ScalarE with the
    1/sqrt(depth) scale folded in (scores bounded, no max subtraction);
    the ScalarE is the bottleneck engine (~141us) and everything else is
    scheduled to hide behind it.
  - PV runs as attn^T = ex^T @ [V|1]: the exp tile is the stationary
    operand, V the moving operand, so each matmul streams only 65
    columns. Each (head, qsub) 16-chunk accumulation chain owns a psum
    ring tile (a psum bank holds one accumulation group at a time and a
    new group wipes the bank), and normalizes immediately: the
    denominator lands on the same partition as its query, so
    normalization is a DVE reciprocal + tensor_scalar multiply.
  - Normalized attn^T subtiles are PE-transposed (f32r + identity) into
    feature-major staging for the collective.
  - The all-to-all (head-split -> row-split) is split into 2 half-batch
    collectives per batch with interleaved row ownership (core j owns
    cols h*1024 + j*128 + [0,128) of each batch), so 3 of the 4
    collectives overlap attention; only the last sits in the tail, with
    a PE<->DVE ping-pong chain keeping the tensor engine in its max
    p-state until the final output projection.
  - Output projection per (batch, half): per output chunk, 8 psum
    accumulations over the 8 feature chunks, bias from the host-folded
    bo' = bo + Wo @ bv.
The emission interleaves a filler queue (next batch's projections, PV
chains, output projection chunks, weight loads) into the ScalarE-paced
attention stream on a ~600ns/iteration budget so the PE never idles
long enough to drop out of its max p-state. Engine assignment respects
hardware limits: GPSIMD never touches PSUM, and the ScalarE runs ONLY
exps — any copy placed on it delays the exp cascade through the score
ring, so all PSUM->SBUF copies and bias-adds live on the DVE.
"""

import os
from collections import deque

import numpy as np

import concourse.bacc as bacc
import concourse.mybir as mybir
import concourse.tile as tile

F32 = mybir.dt.float32
F32R = mybir.dt.float32r
BF16 = mybir.dt.bfloat16
AF = mybir.ActivationFunctionType

P = 128


def build_nc(B=2, S=2048, D=1024, H=16, ncores=8, dbg=False):
    DEP = D // H                 # head depth (64)
    HPC = H // ncores            # heads per core (2)
    FPC = HPC * DEP              # features per core (128)
    R = B * S                    # flattened rows (4096)
    KD = D // P                  # contraction chunks for projections (8)
    RWC = 512                    # row chunk for projections (per batch)
    NRWB = S // RWC              # projection row-chunks per batch (4)
    QCH = 512                    # query columns per block
    NQC = S // QCH               # q blocks per batch (4)
    NKC = S // P                 # key chunks per batch (16)
    NTB = S // P                 # V chunks per batch (16)
    NDO = D // P                 # output-feature chunks (8)
    SC = S // ncores             # rows per core per batch (256)
    # collective split: [qc0+qc1 | qc2+qc3] with per-part interleaved row
    # ownership (core j owns cols off + j*W + [0,W) of each batch)
    PARTS = [(0, 128), (1024, 128)]
    NSUB = QCH // P              # q sub-chunks per q block (4)
    assert FPC == P
    scale = 1.0 / np.sqrt(DEP)

    nc = bacc.Bacc("TRN2", target_bir_lowering=False, debug=False,
                   num_devices=ncores)

    xT = nc.dram_tensor("xT", [D, R], BF16, kind="ExternalInput")
    wqkvT = nc.dram_tensor("wqkvT", [D, 3 * FPC], BF16, kind="ExternalInput")
    bq = nc.dram_tensor("bq", [FPC, 1], F32, kind="ExternalInput")
    woT = nc.dram_tensor("woT", [D, D], BF16, kind="ExternalInput")
    bo = nc.dram_tensor("bo", [P, NDO], F32, kind="ExternalInput")
    ident = nc.dram_tensor("ident", [P, P], F32R, kind="ExternalInput")
    outT = nc.dram_tensor("outT", [D, B * SC], F32, kind="ExternalOutput")

    with tile.TileContext(nc) as tc:
        with (
            tc.tile_pool(name="persist", bufs=1) as persist,
            tc.tile_pool(name="stream", bufs=2) as stream,
            tc.tile_pool(name="work", bufs=2) as work,
            tc.tile_pool(name="ps", bufs=1, space="PSUM") as ps,
        ):
            # ---- resident weights / constants (wo is loaded later: it is
            # only needed by the output projection ~halfway in) ----
            wqkv_sb = persist.tile([P, KD, 3 * FPC], BF16)
            nc.sync.dma_start(
                wqkv_sb[:, :, 0:FPC],
                wqkvT.ap()[:, 0:FPC].rearrange("(ko p) m -> p ko m", p=P))
            bq_sb = persist.tile([FPC, 1], F32)
            bo_sb = persist.tile([P, NDO], F32)
            ident_sb = persist.tile([P, P], F32R)
            wo_sb = persist.tile([P, KD, D], BF16)

            QT = persist.tile([P, R], BF16)
            KT = persist.tile([P, R], BF16)
            # V in [keys, feat] layout: per chunk t cols [0:64]=headA V,
            # col 64 = ones (softmax denominator), [65:129]=headB V,
            # col 129 = ones.
            V_sb = persist.tile([P, B * NTB, 2 * (DEP + 1)], BF16)
            nc.vector.memset(V_sb[:, :, DEP:DEP + 1], 1.0)
            nc.vector.memset(V_sb[:, :, 2 * DEP + 1:2 * DEP + 2], 1.0)

            # feature-major staging for the collective: [feat, q-global]
            stg = [persist.tile([P, S], BF16, name=f"stg_{b}")
                   for b in range(B)]

            a2a_in = [[nc.dram_tensor(f"a2a_in_{b}_{pi}",
                                      [ncores, FPC, w], BF16,
                                      kind="Internal")
                       for pi, (_, w) in enumerate(PARTS)]
                      for b in range(B)]
            a2a_out = [[nc.dram_tensor(f"a2a_out_{b}_{pi}",
                                       [ncores, FPC, w], BF16,
                                       kind="Internal")
                        for pi, (_, w) in enumerate(PARTS)]
                       for b in range(B)]

            warm = persist.tile([P, RWC], BF16, name="warm")
            nc.vector.memset(warm, 0.00390625)
            wps = ps.tile([P, 2 * QCH], F32, tag="sc", bufs=2, name="warmps")
            for i in range(4):
                nc.tensor.matmul(wps[:, 0:RWC], warm[:, 0:P], warm,
                                 start=True, stop=True)

            xs_tiles = {}

            def xs_load(b, rwb, split=False):
                xs = stream.tile([P, KD, RWC], BF16, tag="xs", bufs=6,
                                 name=f"xs_{b}_{rwb}")
                r0 = b * S + rwb * RWC
                src = xT.ap()[:, r0:r0 + RWC].rearrange(
                    "(ko p) n -> p ko n", p=P)
                if split:
                    h = KD // 2
                    nc.sync.dma_start(xs[:, 0:h, :], src[:, 0:h, :])
                    nc.sync.dma_start(xs[:, h:KD, :], src[:, h:KD, :])
                else:
                    nc.sync.dma_start(xs, src)
                xs_tiles[(b, rwb)] = xs

            def proj(b, rwb, j):
                # j: 0=Q (bias on DVE), 1=K (copy: ScalarE for batch 0 —
                # it has slack during the projection phase — DVE for batch 1)
                r0 = b * S + rwb * RWC
                xs = xs_tiles[(b, rwb)]
                if b == 0 and rwb == 0 and j == 0:
                    pq = ps.tile([P, 2 * QCH], F32, tag="sc", bufs=2,
                                 name="pq_first")[:, 0:RWC]
                else:
                    pq = ps.tile([P, RWC], F32, tag="pq", bufs=1,
                                 name=f"pq_{b}_{rwb}_{j}")
                for ko in range(KD):
                    nc.tensor.matmul(
                        pq, wqkv_sb[:, ko, j * FPC:(j + 1) * FPC],
                        xs[:, ko, :], start=(ko == 0), stop=(ko == KD - 1))
                if j == 0:
                    nc.vector.tensor_scalar_add(QT[:, r0:r0 + RWC], pq, bq_sb)
                else:
                    nc.vector.tensor_copy(KT[:, r0:r0 + RWC], pq)

            def vproj(b, rwb):
                # V computed directly in [keys, feat] layout: x^T row-chunk
                # stationary, W_v^T moving, so no transpose is ever needed.
                xs = xs_tiles[(b, rwb)]
                for rc in range(RWC // P):
                    t = b * NTB + rwb * (RWC // P) + rc
                    if b == 0:
                        pq = ps.tile([P, P], F32, tag="attn", bufs=3,
                                     name=f"pv_{b}_{rwb}_{rc}")
                    else:
                        pq = ps.tile([P, RWC], F32, tag="pq", bufs=1,
                                     name=f"pv_{b}_{rwb}_{rc}")
                    for ko in range(KD):
                        nc.tensor.matmul(
                            pq[:, 0:P],
                            xs[:, ko, rc * P:(rc + 1) * P],
                            wqkv_sb[:, ko, 2 * FPC:3 * FPC],
                            start=(ko == 0), stop=(ko == KD - 1))
                    dst = V_sb[:, t, :].rearrange("p (h f) -> p h f", h=HPC)[
                        :, :, 0:DEP]
                    src_ap = pq[:, 0:P].rearrange("p (h f) -> p h f", h=HPC)
                    nc.vector.tensor_copy(dst, src_ap)

            ex_tiles = {}
            attn_tiles = {}

            def scores_exp(b, qc, kc):
                g0 = b * S + qc * QCH
                k0 = b * S + kc * P
                sc = ps.tile([P, 2 * QCH], F32, tag="sc", bufs=2,
                             name=f"sc_{b}_{qc}_{kc}")
                nc.tensor.matmul(
                    sc[:, 0:QCH], KT[0:DEP, k0:k0 + P],
                    QT[0:DEP, g0:g0 + QCH],
                    start=True, stop=True, tile_position=(0, 0))
                nc.tensor.matmul(
                    sc[:, QCH:2 * QCH], KT[DEP:2 * DEP, k0:k0 + P],
                    QT[DEP:2 * DEP, g0:g0 + QCH],
                    start=True, stop=True, tile_position=(DEP, 0))
                ex = work.tile([P, 2 * QCH], BF16, tag="ex", bufs=26,
                               name=f"ex_{b}_{qc}_{kc}")
                nc.scalar.activation(ex, sc, AF.Exp, scale=scale)
                ex_tiles[(b, qc, kc)] = ex

            asb_tiles = {}

            def pv_unit(b, qc, h, qs, act=False):
                # attn^T accumulation: ex chunk stationary, [V|1] moving.
                # Each 16-kc chain gets its own psum ring tile (a psum bank
                # supports one accumulation group at a time, and starting a
                # new group wipes the bank), then normalizes immediately:
                # the denominator lands on the same partition as its query.
                at = ps.tile([P, P], F32, tag="attn", bufs=3,
                             name=f"at_{b}_{qc}_{h}_{qs}")
                for kc in range(NKC):
                    nc.tensor.matmul(
                        at[:, 0:DEP + 1],
                        ex_tiles[(b, qc, kc)][
                            :, h * QCH + qs * P:h * QCH + (qs + 1) * P],
                        V_sb[:, b * NTB + kc,
                             h * (DEP + 1):(h + 1) * (DEP + 1)],
                        start=(kc == 0), stop=(kc == NKC - 1))
                if (b, qc) not in asb_tiles:
                    asb_tiles[(b, qc)] = work.tile(
                        [P, QCH], F32R, tag="asb", bufs=2,
                        name=f"asb_{b}_{qc}")
                asb = asb_tiles[(b, qc)]
                ra = work.tile([P, 1], F32, tag="ra", bufs=8,
                               name=f"ra_{b}_{qc}_{h}_{qs}")
                nc.vector.reciprocal(ra, at[:, DEP:DEP + 1])
                if act:
                    # tail: ScalarE is idle after its last exp — multiply
                    # there so the DVE only does the reciprocals
                    nc.scalar.mul(
                        asb[:, qs * P + h * DEP:qs * P + (h + 1) * DEP],
                        at[:, 0:DEP], ra)
                else:
                    nc.vector.tensor_scalar_mul(
                        asb[:, qs * P + h * DEP:qs * P + (h + 1) * DEP],
                        at[:, 0:DEP], ra)

            def norm_stage(b, qc):
                # stg[f, qc*QCH + c*128 + q] = asb[q, c*128 + f], via PE
                # transposes (DMA transposes serialize against collectives).
                # One psum ring tile per 128-col subtile: a fresh accumulation
                # group would wipe a shared bank's earlier subtiles.
                asb = asb_tiles.pop((b, qc))
                for c in range(NSUB):
                    tr = ps.tile([P, RWC], F32R, tag="pq", bufs=1,
                                 name=f"tr_{b}_{qc}_{c}")
                    nc.tensor.transpose(tr[:, 0:P],
                                        asb[:, c * P:(c + 1) * P], ident_sb)
                    nc.vector.tensor_copy(
                        stg[b][:, qc * QCH + c * P:qc * QCH + (c + 1) * P],
                        tr[:, 0:P])

            def stage_dma(b, pi, jlo=0, jhi=None):
                off, w = PARTS[pi]
                jhi = ncores if jhi is None else jhi
                nc.sync.dma_start(
                    a2a_in[b][pi].ap()[jlo:jhi].rearrange("j p q -> p j q"),
                    stg[b][:, off + jlo * w:off + jhi * w].rearrange(
                        "p (j q) -> p j q", j=jhi - jlo))

            def collective(b, pi):
                nc.gpsimd.collective_compute(
                    "AllToAll", mybir.AluOpType.bypass,
                    replica_groups=[list(range(ncores))],
                    ins=[a2a_in[b][pi].ap().opt()],
                    outs=[a2a_out[b][pi].ap().opt()])

            ca_tiles = {}

            def ca_load(b, pi, split=False):
                w = PARTS[pi][1]
                ca = stream.tile([P, KD, w], BF16, tag=f"ca{w}", bufs=2,
                                 name=f"ca_{b}_{pi}")
                src = a2a_out[b][pi].ap().rearrange("j p q -> p j q")
                if split:
                    h = KD // 2
                    nc.sync.dma_start(ca[:, 0:h, :], src[:, 0:h, :])
                    nc.sync.dma_start(ca[:, h:KD, :], src[:, h:KD, :])
                else:
                    nc.sync.dma_start(ca, src)
                ca_tiles[(b, pi)] = ca

            otb_tiles = {}

            OUT_OFF = [0, 128]    # outT col offset of each part

            def out_proj_do(b, pi, do, alt=False):
                w = PARTS[pi][1]
                ca = ca_tiles[(b, pi)]
                if (b, pi) not in otb_tiles:
                    otb_tiles[(b, pi)] = work.tile(
                        [P, NDO, w], F32, tag=f"otb{w}", bufs=2,
                        name=f"otb_{b}_{pi}")
                otb = otb_tiles[(b, pi)]
                if alt and do % 2:
                    op = ps.tile([P, 2 * QCH], F32, tag="sc", bufs=2,
                                 name=f"op_{b}_{pi}_{do}")
                else:
                    op = ps.tile([P, RWC], F32, tag="pq", bufs=1,
                                 name=f"op_{b}_{pi}_{do}")
                for j in range(KD):
                    nc.tensor.matmul(
                        op[:, 0:w], wo_sb[:, j, do * P:(do + 1) * P],
                        ca[:, j, :], start=(j == 0), stop=(j == KD - 1))
                nc.vector.tensor_scalar_add(
                    otb[:, do, :], op[:, 0:w], bo_sb[:, do:do + 1])

            def out_dma(b, pi):
                w = PARTS[pi][1]
                c0 = b * SC + OUT_OFF[pi]
                nc.sync.dma_start(
                    outT.ap()[:, c0:c0 + w]
                        .rearrange("(dd p) n -> p dd n", p=P),
                    otb_tiles.pop((b, pi)))

            # ---- filler machinery: (cost_ns, fn) drained into the
            # ACT-paced attention stream at ~budget ns per kc iteration ----
            fillers = deque()

            def drain(budget):
                while fillers and fillers[0][0] <= budget:
                    cost, fn = fillers.popleft()
                    fn()
                    budget -= cost
                return budget

            def attn_qc(b, qc, start_kc=0, per_iter=420, tail_pv=True):
                budget = 0
                for kc in range(start_kc, NKC):
                    scores_exp(b, qc, kc)
                    budget = drain(budget + per_iter)
                if tail_pv:
                    # head B chains inline (their last matmuls overlap the
                    # final exps); head A chains go in as early fillers of
                    # the next window via pv_fillers()
                    for qs in range(NSUB):
                        pv_unit(b, qc, 1, qs)

            def pv_fillers(b, qc):
                for qs in range(NSUB):
                    fillers.append((450, lambda qs=qs: pv_unit(b, qc, 0, qs)))
                fillers.append((400, lambda: norm_stage(b, qc)))

            # ================= emission =================
            # phase A: batch 0 projections + qc0 scores + qc1 head-start
            # (K/V weight columns load after the first x chunk: only the
            # Q columns gate the first projection)
            for rwb in range(NRWB):
                xs_load(0, rwb, split=(rwb == 0))
                if rwb == 0:
                    nc.sync.dma_start(
                        wqkv_sb[:, :, FPC:2 * FPC],
                        wqkvT.ap()[:, FPC:2 * FPC].rearrange(
                            "(ko p) m -> p ko m", p=P))
                    nc.sync.dma_start(bq_sb, bq.ap())
                    nc.sync.dma_start(
                        wqkv_sb[:, :, 2 * FPC:3 * FPC],
                        wqkvT.ap()[:, 2 * FPC:3 * FPC].rearrange(
                            "(ko p) m -> p ko m", p=P))
                    nc.sync.dma_start(bo_sb, bo.ap())
                    nc.sync.dma_start(ident_sb, ident.ap())
                    # rwb0: Q first (every qc0 score needs it)
                    proj(0, rwb, 0)
                    proj(0, rwb, 1)
                else:
                    # rwb>=1: K first — this chunk's scores need only its K
                    # (their Q block is from earlier chunks); Q can lag
                    proj(0, rwb, 1)
                for kc in range(rwb * (NKC // NRWB), (rwb + 1) * (NKC // NRWB)):
                    scores_exp(0, 0, kc)
                if rwb > 0:
                    proj(0, rwb, 0)
                vproj(0, rwb)
                if rwb == 2:
                    for kc in range(0, 4):
                        scores_exp(0, 1, kc)
                if rwb == 3:
                    for kc in range(4, 8):
                        scores_exp(0, 1, kc)

            # qc1 (its first 8 scores ran in phase A); qc0's PV chains and
            # batch-1's first V projection drain as fillers
            for h in (1, 0):
                for qs in range(NSUB):
                    fillers.append((450, lambda h=h, qs=qs: pv_unit(0, 0, h, qs)))
            fillers.append((400, lambda: norm_stage(0, 0)))
            fillers.append((0, lambda: xs_load(1, 0)))
            fillers.append((1700, lambda: vproj(1, 0)))
            attn_qc(0, 1, start_kc=8, per_iter=640)

            # qc2: batch-1 V projections + both V DMA-transposes (these must
            # be emitted before the first collective), then part-0 staging
            pv_fillers(0, 1)
            fillers.append((0, lambda: xs_load(1, 1)))
            fillers.append((1700, lambda: vproj(1, 1)))
            fillers.append((0, lambda: xs_load(1, 2)))
            fillers.append((1700, lambda: vproj(1, 2)))
            fillers.append((0, lambda: stage_dma(0, 0)))
            fillers.append((0, lambda: collective(0, 0)))
            fillers.append((1700, lambda: proj(1, 0, 1)))
            attn_qc(0, 2, per_iter=600)

            # qc3: batch-1 K copies + first Q (all flushed before the b1
            # windows so b1 scores never precede their QT/KT writes)
            pv_fillers(0, 2)
            fillers.append((1700, lambda: proj(1, 1, 1)))
            fillers.append((1700, lambda: proj(1, 2, 1)))
            fillers.append((1700, lambda: proj(1, 0, 0)))
            attn_qc(0, 3, per_iter=600)
            while fillers:
                fillers.popleft()[1]()

            # batch 1 attention
            pv_fillers(0, 3)
            fillers.append((0, lambda: stage_dma(0, 1)))
            fillers.append((0, lambda: collective(0, 1)))
            fillers.append((0, lambda: xs_load(1, 3)))
            fillers.append((1700, lambda: vproj(1, 3)))
            fillers.append((1700, lambda: proj(1, 3, 1)))
            fillers.append((1700, lambda: proj(1, 1, 0)))
            attn_qc(1, 0, per_iter=640)
            while fillers:
                fillers.popleft()[1]()

            pv_fillers(1, 0)
            fillers.append((1700, lambda: proj(1, 2, 0)))
            fillers.append((1700, lambda: proj(1, 3, 0)))
            for jc in range(KD):
                fillers.append((80, lambda jc=jc: nc.sync.dma_start(
                    wo_sb[:, jc, :],
                    woT.ap()[jc * P:(jc + 1) * P, :])))
            attn_qc(1, 1, per_iter=620)
            while fillers:
                fillers.popleft()[1]()

            pv_fillers(1, 1)
            fillers.append((0, lambda: stage_dma(1, 0)))
            fillers.append((0, lambda: collective(1, 0)))
            fillers.append((0, lambda: ca_load(0, 0)))
            for do in range(NDO):
                fillers.append((560, lambda do=do: out_proj_do(0, 0, do)))
            fillers.append((0, lambda: out_dma(0, 0)))
            attn_qc(1, 2, per_iter=620)

            pv_fillers(1, 2)
            fillers.append((0, lambda: stage_dma(1, 1, 0, 4)))
            fillers.append((0, lambda: ca_load(0, 1)))
            for do in range(NDO):
                fillers.append((560, lambda do=do: out_proj_do(0, 1, do)))
            fillers.append((0, lambda: out_dma(0, 1)))
            attn_qc(1, 3, per_iter=620, tail_pv=False)
            while fillers:
                fillers.popleft()[1]()
            # interleaved tail: per qsub both heads' PV chains, then that
            # subtile's staging transpose, so the last collective launches
            # as early as possible
            for qs in range(NSUB):
                pv_unit(1, 3, 1, qs)
                pv_unit(1, 3, 0, qs)
                asb = asb_tiles[(1, 3)]
                tr = ps.tile([P, RWC], F32R, tag="pq", bufs=1,
                             name=f"ttr_{qs}")
                nc.tensor.transpose(tr[:, 0:P], asb[:, qs * P:(qs + 1) * P],
                                    ident_sb)
                nc.vector.tensor_copy(
                    stg[1][:, 3 * QCH + qs * P:3 * QCH + (qs + 1) * P],
                    tr[:, 0:P])
                stage_dma(1, 1, 4 + qs, 5 + qs)
            asb_tiles.pop((1, 3))
            collective(1, 1)
            # part 0 of batch 1 landed during attention; its output
            # projection (plus p-state keep-alive matmuls) overlaps the tail
            # collective so part 1's projection runs at full clock
            ca_load(1, 0)
            for do in range(NDO):
                out_proj_do(1, 0, do, alt=True)
            out_dma(1, 0)
            # ping-pong PE<->DVE chain paces ~1us/round so the PE never
            # idles long enough to drop out of its max p-state while the
            # tail collective is in flight
            for i in range(43):
                ramp = ps.tile([P, 2 * QCH], F32, tag="sc", bufs=2,
                               name=f"ramp_{i}")
                nc.tensor.matmul(ramp[:, 0:P], warm[:, 0:P], warm[:, 0:P],
                                 start=True, stop=True)
                nc.vector.tensor_copy(warm[:, 0:P].bitcast(F32)[:, 0:64],
                                      ramp[:, 0:64])
            ca_load(1, 1, split=True)
            for do in range(NDO):
                out_proj_do(1, 1, do, alt=True)
            out_dma(1, 1)

    nc.finalize()
    return nc


# ---------------- host side ----------------

_NC_CACHE = {}

B, S, D, H = 2, 2048, 1024, 16
NCORES = 8


def _prep_inputs(x, Wq, bq, Wk, bk, Wv, bv, Wo, bo, ncores):
    import ml_dtypes
    bf16 = ml_dtypes.bfloat16
    Dl = x.shape[-1]
    R = x.shape[0] * x.shape[1]
    FPC = Dl // ncores
    NDO = Dl // P
    xT = np.ascontiguousarray(x.reshape(R, Dl).T.astype(bf16))
    woT = np.ascontiguousarray(Wo.T.astype(bf16))
    # fold bv into bo: out = (attn0 + bv) @ Wo^T + bo
    bo_f = bo + Wo @ bv
    bo2 = np.ascontiguousarray(bo_f.reshape(NDO, P).T.astype(np.float32))
    identm = np.eye(P, dtype=np.float32)
    maps = []
    for c in range(ncores):
        fsl = slice(c * FPC, (c + 1) * FPC)
        wqkvT = np.ascontiguousarray(
            np.concatenate([Wq[fsl], Wk[fsl], Wv[fsl]], axis=0).T.astype(bf16))
        bqc = np.ascontiguousarray(
            bq[fsl].reshape(FPC, 1).astype(np.float32))
        maps.append(dict(xT=xT, wqkvT=wqkvT, bq=bqc, woT=woT, bo=bo2,
                         ident=identm))
    return maps


def kernel(x, Wq, bq, Wk, bk, Wv, bv, Wo, bo):
    from concourse.bass_utils import run_bass_kernel_spmd

    args = [np.asarray(a, np.float32)
            for a in (x, Wq, bq, Wk, bk, Wv, bv, Wo, bo)]
    x = args[0]
    Bx, Sx, Dx = x.shape
    key = (Bx, Sx, Dx)
    if key not in _NC_CACHE:
        _NC_CACHE[key] = build_nc(B=Bx, S=Sx, D=Dx, H=H, ncores=NCORES)
    nc = _NC_CACHE[key]

    in_maps = _prep_inputs(*args, NCORES)
    trace = os.environ.get("KERNEL_TRACE", "0") == "1"
    try:
        res = run_bass_kernel_spmd(nc, in_maps, core_ids=list(range(NCORES)),
                                   trace=trace)
    except ModuleNotFoundError:
        res = run_bass_kernel_spmd(nc, in_maps, core_ids=list(range(NCORES)),
                                   trace=False)
    kernel._last_results = res
    Sc = Sx // NCORES          # 256
    PARTS_H = [(0, 128), (1024, 128)]
    OUT_OFF_H = [0, 128]
    out = np.empty((Bx * Sx, Dx), np.float32)
    for c in range(NCORES):
        oc = res.results[c]["outT"].T  # [B*Sc, D]; cols (b, part) interleaved
        for b2 in range(Bx):
            for (off, w), oo in zip(PARTS_H, OUT_OFF_H):
                r0 = b2 * Sx + off + c * w
                out[r0:r0 + w] = oc[b2 * Sc + oo:b2 * Sc + oo + w]
    return np.ascontiguousarray(out).reshape(Bx, Sx, Dx)
